# revision 30
# baseline (speedup 1.0000x reference)
"""MultiHeadAttention TRN2 Bass kernel (final, ~239us vs 410us v1 baseline).

Full-input contract: kernel(**inputs) takes the unsharded tensors from
setup_inputs() and returns the full [4, 2048, 512] output.

Sharding: 8 cores = 4 batches x 2 query-halves. Each core computes its own
[1024, 512] slice of the output for one batch over all 8 heads; the gather
is a pure concatenation (no collectives).

Design (trace-driven, see per-phase notes inline):
  - Host pre-transposes q/k/v slices and all weights (W.T) and casts to
    bf16, so activations/weights DMA straight into matmul-ready SBUF
    layouts: no PE transposes, no f32r cast passes (v1 spent ~100us there).
  - Whole pipeline bf16 with fp32 PSUM accumulation; rel err 3.4e-3 vs
    the 2e-2 gate. (fp8+DoubleRow AV was measured ~9us faster but lands at
    rel err 1.95e-2 -- too close to the gate to ship.)
  - Attention runs per head-pair, ACT-bound: per k-chunk the two heads'
    score matmuls (rows 0-63 / 64-127 via auto tile_position) write two
    2-bank psum tiles ping-ponged against [128,1024] exp calls, keeping
    the scalar engine ~88% busy; exp output feeds a 4-slot bf16 ring.
  - AV (with fused ones-column for free rowsums) lags 2 k-chunks and the
    leftover groups are injected between the next pair's first score
    blocks, so neither PE nor ACT idles across pair boundaries (HAM
    re-throttles to 1.2GHz on PE duty dips; v1 ran its whole attention
    phase at half clock because 64-row score matmuls never read as busy).
  - Input DMA is one need-ordered queue (wq,xq | wk,xk | wv,xv | rest):
    splitting across queues just divides bandwidth and stalls the PE.
  - Tail: rowsums packed at 32-aligned partitions, reciprocal as
    exp(-ln(x)) on the idle ACT engine (same table set as the attention
    exps), selector-matmul broadcast, all pipelined by q-half so the out
    projection of the first half overlaps the second half's normalization.
"""
import contextlib

import numpy as np

import bass_rust
import concourse.bass as bass
import concourse.mybir as mybir
import concourse.tile as tile
from concourse.bass_utils import run_bass_kernel_spmd
from concourse.tile import add_dep_helper

F32 = mybir.dt.float32
F32R = mybir.dt.float32r
BF16 = mybir.dt.bfloat16

B, S, D_MODEL = 4, 2048, 512
NUM_HEADS = 8
HEAD_DIM = 64
SQ = S // 2  # queries per core
N_CORES = 8
SCALE = 1.0 / 8.0  # 1/sqrt(HEAD_DIM)
KTILES = S // 128  # 16 k-chunks
H2 = NUM_HEADS // 2  # head pairs
RING = 4  # eh ring depth (k-chunks buffered between exp and AV)

_split_ctr = [0]


def split_waits(nc, max_waits: int = 1):
    """walrus codegen rejects instructions carrying >1 sync wait; move the
    extras onto standalone EventSemaphore instructions on the same engine."""
    for f in nc.m.functions:
        for blk in f.blocks:
            new_insts = []
            changed = False
            for inst in blk.instructions:
                si = inst.sync_info
                if si is not None and si.on_wait and len(si.on_wait) > max_waits:
                    waits = list(si.on_wait)
                    extra, keep = waits[:-max_waits], waits[-max_waits:]
                    for w in extra:
                        _split_ctr[0] += 1
                        ev = mybir.InstEventSemaphore(
                            name=f"I-wsplit-{_split_ctr[0]}", ins=[], outs=[]
                        )
                        ev.engine = inst.engine
                        ev.sync_info = bass_rust.SyncInfo(on_wait=[w], on_update=[])
                        new_insts.append(ev)
                    inst.sync_info = bass_rust.SyncInfo(
                        on_wait=keep, on_update=list(si.on_update)
                    )
                    changed = True
                new_insts.append(inst)
            if changed:
                blk.instructions = new_insts


def build_mha():
    nc = bass.Bass("TRN2", target_bir_lowering=False, debug=False, num_devices=1)

    # host-transposed inputs: xq = q_slice.T [D, SQ], xk = k.T, xv = v.T,
    # weights passed as W.T [d_in, d_out]; all bf16.
    xqd = nc.declare_dram_parameter("xq", [128, 4, SQ], BF16, isOutput=False).ap()
    xkd = nc.declare_dram_parameter("xk", [128, 4, S], BF16, isOutput=False).ap()
    xvd = nc.declare_dram_parameter("xv", [128, 4, S], BF16, isOutput=False).ap()
    wts = {
        n: nc.declare_dram_parameter(n, [128, 4, D_MODEL], BF16, isOutput=False).ap()
        for n in ("wq", "wk", "wv", "wo")
    }
    bias = {
        n: nc.declare_dram_parameter(n, [D_MODEL], F32, isOutput=False).ap()
        for n in ("bq", "bk", "bv", "bo")
    }
    outd = nc.declare_dram_parameter("out", [SQ, D_MODEL], F32, isOutput=True).ap()

    # selector rows for the rowsum-reciprocal broadcast: selU row 32t maps
    # head 2t's reciprocal to PE cols 0-63, selD row 32t to cols 64-127
    # (head 2t+1). Rows sit at 32-aligned partitions: DVE/matmul APs must
    # start at a multiple of 32.
    selu_np = np.zeros((128, 128), np.float32)
    seld_np = np.zeros((128, 128), np.float32)
    for t in range(H2):
        selu_np[32 * t, 0:HEAD_DIM] = 1.0
        seld_np[32 * t, HEAD_DIM:128] = 1.0
    selu_dram = nc.inline_tensor(selu_np, name="selu_const")
    seld_dram = nc.inline_tensor(seld_np, name="seld_const")

    with tile.TileContext(nc) as tc, contextlib.ExitStack() as top:
        consts = top.enter_context(tc.tile_pool(name="consts", bufs=1))
        wt_pool = top.enter_context(tc.tile_pool(name="wt", bufs=1))
        proj_out = top.enter_context(tc.tile_pool(name="proj_out", bufs=1))
        epilog = top.enter_context(tc.tile_pool(name="epilog", bufs=1))

        # ---- constants
        bqt = consts.tile([128, 4], F32)
        bkt = consts.tile([128, 4], F32)
        for t_, name in ((bqt, "bq"), (bkt, "bk")):
            nc.gpsimd.dma_start(
                out=t_, in_=bias[name].rearrange("(c p) -> p c", p=128)
            )
        bvb = consts.tile([128, D_MODEL], F32)
        bob = consts.tile([128, D_MODEL], F32)
        ones8 = consts.tile([128, NUM_HEADS], BF16)
        nc.vector.memset(ones8, 1.0)
        selu_f = consts.tile([128, 128], F32)
        seld_f = consts.tile([128, 128], F32)
        selu = consts.tile([128, 128], F32R)
        seld = consts.tile([128, 128], F32R)
        # warm the ACT exp table early (table load ~2.7us)
        dmy = consts.tile([1, 8], F32)
        nc.vector.memset(dmy, 0.0)
        dmy2 = consts.tile([1, 8], F32)
        nc.scalar.activation(dmy2, dmy, mybir.ActivationFunctionType.Exp)

        # ---- persistent weight / activation tiles
        WSB = {
            n: wt_pool.tile([128, 4, D_MODEL], BF16, name=f"w_{n}", tag=f"w_{n}")
            for n in wts
        }
        QT = [proj_out.tile([128, SQ], BF16, name=f"qt_{t}", tag=f"qt_{t}") for t in range(H2)]
        KT = [proj_out.tile([128, S], BF16, name=f"kt_{t}", tag=f"kt_{t}") for t in range(H2)]
        V = [
            proj_out.tile([128, NUM_HEADS, HEAD_DIM + 1], BF16, name=f"v_{sc}", tag=f"v_{sc}")
            for sc in range(KTILES)
        ]
        OU = [epilog.tile([128, SQ], BF16, name=f"ou_{t}", tag=f"ou_{t}") for t in range(H2)]
        # rowsums: head 2t at RSA[32t], head 2t+1 at RSB[32t] (32-aligned)
        RSA = epilog.tile([128, SQ], F32, name="rsa", tag="rsa")
        RSB = epilog.tile([128, SQ], F32, name="rsb", tag="rsb")
        RSAr = epilog.tile([128, SQ], F32R, name="rsar", tag="rsar")
        RSBr = epilog.tile([128, SQ], F32R, name="rsbr", tag="rsbr")
        LNS = epilog.tile([128, SQ], F32, name="lns", tag="lns")
        OMT = [epilog.tile([128, SQ], BF16, name=f"omt_{t}", tag=f"omt_{t}") for t in range(H2)]
        nc.vector.memset(RSA, 1.0)  # unused rows must not hit 1/0
        nc.vector.memset(RSB, 1.0)

        pe_chain = [None]

        def chain(bi):
            if pe_chain[0] is not None:
                add_dep_helper(bi.ins, pe_chain[0].ins, reason="pe-order")
            pe_chain[0] = bi

        # ================= phase 1: projections ===========================
        with (
            tc.tile_pool(name="xin", bufs=1) as xin,
            tc.tile_pool(name="pp", bufs=2, space="PSUM") as pp,
        ):
            xq_sb = xin.tile([128, 4, SQ], BF16, tag="xq")
            xk_sb = xin.tile([128, 4, S], BF16, tag="xk")
            xv_sb = xin.tile([128, 4, S], BF16, tag="xv")
            # single need-ordered queue; tensors are host-packed into the
            # exact SBUF layout, so each DMA is one contiguous run per
            # partition (the strided per-chunk loads were descriptor-bound:
            # first matmul waited ~9us for 1.5MB)
            nc.sync.dma_start(out=WSB["wq"], in_=wts["wq"])
            nc.sync.dma_start(out=xq_sb, in_=xqd)
            nc.sync.dma_start(out=WSB["wk"], in_=wts["wk"])
            nc.sync.dma_start(out=xk_sb, in_=xkd)
            nc.sync.dma_start(out=WSB["wv"], in_=wts["wv"])
            nc.sync.dma_start(out=xv_sb, in_=xvd)
            nc.sync.dma_start(out=WSB["wo"], in_=wts["wo"])
            # HAM pre-warmer: dense full-array matmuls on memset data keep
            # the PE busy during the DMA shadow so the real projections
            # start at 2.4GHz instead of ramping from 1.2GHz
            warm_sb = xin.tile([128, 512], BF16, tag="warm")
            nc.vector.memset(warm_sb, 0.5)
            warm_ps = pp.tile([128, 512], F32, tag="pproj")
            for _ in range(22):
                chain(
                    nc.tensor.matmul(
                        warm_ps, warm_sb[:, 0:128], warm_sb, start=True, stop=True
                    )
                )
            for t_, name in ((bvb, "bv"), (bob, "bo")):
                bsrc = bias[name]
                nc.scalar.dma_start(
                    out=t_,
                    in_=bass.AP(
                        tensor=bsrc.tensor,
                        offset=bsrc.offset,
                        ap=[[0, 128], [1, D_MODEL]],
                    ),
                )
            nc.scalar.dma_start(out=selu_f, in_=selu_dram.ap())
            nc.scalar.dma_start(out=seld_f, in_=seld_dram.ap())
            nc.vector.tensor_copy(selu, selu_f)
            nc.vector.tensor_copy(seld, seld_f)

            # Q^T / K^T projections: [d_out-chunk t partitions, seq free]
            for wname, bt, xsb, dst, slen in (
                ("wq", bqt, xq_sb, QT, SQ),
                ("wk", bkt, xk_sb, KT, S),
            ):
                for t in range(H2):
                    for qc in range(slen // 512):
                        pj = pp.tile([128, 512], F32, tag="pproj")
                        for dc in range(4):
                            chain(
                                nc.tensor.matmul(
                                    pj,
                                    WSB[wname][:, dc, t * 128 : (t + 1) * 128],
                                    xsb[:, dc, qc * 512 : (qc + 1) * 512],
                                    start=(dc == 0),
                                    stop=(dc == 3),
                                )
                            )
                        nc.vector.tensor_scalar_add(
                            dst[t][:, qc * 512 : (qc + 1) * 512],
                            pj,
                            bt[:, t : t + 1],
                        )

            # V projection: natural [seq partitions, d_out free] + ones col
            for sc in range(KTILES):
                pj = pp.tile([128, 512], F32, tag="pproj")
                for dc in range(4):
                    chain(
                        nc.tensor.matmul(
                            pj,
                            xv_sb[:, dc, sc * 128 : (sc + 1) * 128],
                            WSB["wv"][:, dc, :],
                            start=(dc == 0),
                            stop=(dc == 3),
                        )
                    )
                pj3 = pj.rearrange("p (h d) -> p h d", h=NUM_HEADS)
                nc.vector.tensor_add(
                    V[sc][:, :, 0:HEAD_DIM],
                    pj3,
                    bvb.rearrange("p (h d) -> p h d", h=NUM_HEADS),
                )
                nc.vector.tensor_copy(
                    V[sc][:, :, HEAD_DIM : HEAD_DIM + 1],
                    ones8.rearrange("p (h o) -> p h o", o=1),
                )

        # ================= phase 2: attention (head pairs) ================
        with (
            tc.tile_pool(name="ehpool", bufs=4) as ehpool,
            tc.tile_pool(name="ps_s", bufs=2, space="PSUM") as ps_s,
            tc.tile_pool(name="ps_o", bufs=2, space="PSUM") as ps_o,
        ):
            # dummy allocation shifts the ps_s rotation so pair 0's first
            # score tile lands on the banks NOT written by the final
            # V-projection group (saves a ~2.4us drain wait at the
            # phase transition; psB runs an exp-slot later, by which time
            # the overlapping banks have drained)
            ps_shift = ps_s.tile([128, SQ], F32, tag="ps", name="ps_shift")
            # pending[0] emits the previous pair's last AV group + finalize;
            # it runs between the next pair's first psA and psB score blocks
            # so the ACT engine never idles across the pair boundary.
            pending = [None]
            for t in range(H2):
                poA = ps_o.tile([HEAD_DIM + 1, SQ], F32, tag="po")
                poB = ps_o.tile([HEAD_DIM + 1, SQ], F32, tag="po")
                ehA = ehpool.tile([128, RING, SQ], BF16, tag="eh")
                ehB = ehpool.tile([128, RING, SQ], BF16, tag="eh")

                def emit_av(kc, t=t, poA=poA, poB=poB, ehA=ehA, ehB=ehB):
                    for ph, po_, eh_ in ((2 * t, poA, ehA), (2 * t + 1, poB, ehB)):
                        for qc in range(2):
                            sl = slice(qc * 512, (qc + 1) * 512)
                            chain(
                                nc.tensor.matmul(
                                    po_[:, sl],
                                    V[kc][:, ph, :],
                                    eh_[:, kc % RING, sl],
                                    start=(kc == 0),
                                    stop=(kc == KTILES - 1),
                                )
                            )

                streams = ((slice(0, HEAD_DIM), ehA), (slice(HEAD_DIM, 128), ehB))
                for kc in range(KTILES):
                    ps_tiles = []
                    for si, (rows, _eh) in enumerate(streams):
                        ps_ = ps_s.tile([128, SQ], F32, tag="ps")
                        ps_tiles.append(ps_)
                        for qc in range(2):
                            sl = slice(qc * 512, (qc + 1) * 512)
                            chain(
                                nc.tensor.matmul(
                                    ps_[:, sl],
                                    KT[t][rows, kc * 128 : (kc + 1) * 128],
                                    QT[t][rows, sl],
                                    start=True,
                                    stop=True,
                                )
                            )
                        # inject the previous pair's tail between the two
                        # score blocks: psB is gated by its exp anyway, so
                        # the AV matmuls fill that PE slot for free
                        if si == 0 and pending[0] is not None:
                            if kc == 0:
                                pending[0][0]()  # prev pair AV(14)
                            elif kc == 1:
                                pending[0][1]()  # prev pair AV(15) + finalize
                    for (rows, eh_), ps_ in zip(streams, ps_tiles):
                        nc.scalar.activation(
                            eh_[:, kc % RING, :],
                            ps_,
                            mybir.ActivationFunctionType.Exp,
                            scale=SCALE,
                        )
                    if kc > 1:
                        emit_av(kc - 2)

                def pair_tail_a(emit_av=emit_av):
                    emit_av(KTILES - 2)

                def pair_tail_b(t=t, poA=poA, poB=poB, emit_av=emit_av):
                    emit_av(KTILES - 1)
                    # rowsum copies first: they gate the tail's Ln/Exp chain
                    nc.vector.tensor_copy(
                        RSA[32 * t : 32 * t + 1, :], poA[HEAD_DIM : HEAD_DIM + 1, :]
                    )
                    nc.vector.tensor_copy(
                        RSB[32 * t : 32 * t + 1, :], poB[HEAD_DIM : HEAD_DIM + 1, :]
                    )
                    nc.vector.tensor_copy(OU[t][0:HEAD_DIM, :], poA[0:HEAD_DIM, :])
                    nc.vector.tensor_copy(OU[t][HEAD_DIM:128, :], poB[0:HEAD_DIM, :])

                pending[0] = (pair_tail_a, pair_tail_b)
            pending[0][0]()
            pending[0][1]()

        # ================= phase 3: normalize + out projection ============
        with (
            tc.tile_pool(name="outsb", bufs=3) as outsb,
            tc.tile_pool(name="ps_n", bufs=3, space="PSUM") as ps_n,
            tc.tile_pool(name="ps_f", bufs=2, space="PSUM") as ps_f,
        ):
            # 1/x as exp(-ln(x)) on the otherwise-idle ACT engine (Ln and Exp
            # share the natural_log_exp table set; DVE reciprocal would cost
            # ~6.5us per tile). The whole tail is pipelined by q-half: the
            # out-projection for q-chunks 0-3 only needs the qc=0 half of
            # the normalization, so it starts while qc=1 is still on ACT.
            Ln, Exp = mybir.ActivationFunctionType.Ln, mybir.ActivationFunctionType.Exp

            for qc in range(2):
                sl = slice(qc * 512, (qc + 1) * 512)
                nc.scalar.activation(LNS[:, sl], RSA[:, sl], Ln)
                nc.scalar.activation(RSAr[:, sl], LNS[:, sl], Exp, scale=-1.0)
                nc.scalar.activation(LNS[:, sl], RSB[:, sl], Ln)
                nc.scalar.activation(RSBr[:, sl], LNS[:, sl], Exp, scale=-1.0)
                for t in range(H2):
                    rsb = ps_n.tile([128, 512], F32, tag="rsb")
                    chain(
                        nc.tensor.matmul(
                            rsb,
                            selu[32 * t : 32 * t + 1, :],
                            RSAr[32 * t : 32 * t + 1, sl],
                            start=True,
                            stop=False,
                            tile_position=(32 * t, 0),
                        )
                    )
                    chain(
                        nc.tensor.matmul(
                            rsb,
                            seld[32 * t : 32 * t + 1, :],
                            RSBr[32 * t : 32 * t + 1, sl],
                            start=False,
                            stop=True,
                            tile_position=(32 * t, 0),
                        )
                    )
                    nc.vector.tensor_mul(OMT[t][:, sl], OU[t][:, sl], rsb)
                for sq in range(qc * 4, qc * 4 + 4):
                    pf = ps_f.tile([128, D_MODEL], F32, tag="pf")
                    for t in range(H2):
                        chain(
                            nc.tensor.matmul(
                                pf,
                                OMT[t][:, sq * 128 : (sq + 1) * 128],
                                WSB["wo"][:, t, :],
                                start=(t == 0),
                                stop=(t == H2 - 1),
                            )
                        )
                    ot = outsb.tile([128, D_MODEL], F32, tag="ot")
                    nc.vector.tensor_add(ot, pf, bob)
                    nc.sync.dma_start(out=outd[sq * 128 : (sq + 1) * 128, :], in_=ot)

    split_waits(nc)
    return nc


_cached_nc = None


def _get_nc():
    global _cached_nc
    if _cached_nc is None:
        _cached_nc = build_mha()
    return _cached_nc


def make_in_maps(q, k, v, Wq, bq, Wk, bk, Wv, bv, Wo, bo):
    import ml_dtypes

    BF = ml_dtypes.bfloat16
    q = np.asarray(q, dtype=np.float32)
    k = np.asarray(k, dtype=np.float32)
    v = np.asarray(v, dtype=np.float32)
    weights = {
        "wq": np.ascontiguousarray(np.asarray(Wq, np.float32).T.astype(BF)),
        "wk": np.ascontiguousarray(np.asarray(Wk, np.float32).T.astype(BF)),
        "wv": np.ascontiguousarray(np.asarray(Wv, np.float32).T.astype(BF)),
        "wo": np.ascontiguousarray(np.asarray(Wo, np.float32).T.astype(BF)),
        "bq": np.ascontiguousarray(np.asarray(bq, np.float32)),
        "bk": np.ascontiguousarray(np.asarray(bk, np.float32)),
        "bv": np.ascontiguousarray(np.asarray(bv, np.float32)),
        "bo": np.ascontiguousarray(np.asarray(bo, np.float32)),
    }
    def pack(xt):
        # [512, n] -> [128, 4, n] so SBUF partition p holds rows
        # {p, 128+p, 256+p, 384+p} as one contiguous run
        return np.ascontiguousarray(xt.reshape(4, 128, -1).transpose(1, 0, 2))

    weights = {k_: (pack(w) if k_.startswith("w") else w) for k_, w in weights.items()}
    in_maps = []
    for core in range(N_CORES):
        b, qh = core // 2, core % 2
        in_maps.append(
            {
                "xq": pack(q[b, qh * SQ : (qh + 1) * SQ, :].T.astype(BF)),
                "xk": pack(k[b].T.astype(BF)),
                "xv": pack(v[b].T.astype(BF)),
                **weights,
            }
        )
    return in_maps


def kernel(q, k, v, mask, Wq, bq, Wk, bk, Wv, bv, Wo, bo, **_unused):
    in_maps = make_in_maps(q, k, v, Wq, bq, Wk, bk, Wv, bv, Wo, bo)
    nc = _get_nc()
    res = run_bass_kernel_spmd(nc, in_maps, list(range(N_CORES)))
    out = np.empty((B, S, D_MODEL), dtype=np.float32)
    for core in range(N_CORES):
        b, qh = core // 2, core % 2
        out[b, qh * SQ : (qh + 1) * SQ, :] = res.results[core]["out"]
    return out


# revision 32
# speedup vs baseline: 1.0099x; 1.0099x over previous
"""MultiHeadAttention TRN2 Bass kernel (final, ~239us vs 410us v1 baseline).

Full-input contract: kernel(**inputs) takes the unsharded tensors from
setup_inputs() and returns the full [4, 2048, 512] output.

Sharding: 8 cores = 4 batches x 2 query-halves. Each core computes its own
[1024, 512] slice of the output for one batch over all 8 heads; the gather
is a pure concatenation (no collectives).

Design (trace-driven, see per-phase notes inline):
  - Host pre-transposes q/k/v slices and all weights (W.T) and casts to
    bf16, so activations/weights DMA straight into matmul-ready SBUF
    layouts: no PE transposes, no f32r cast passes (v1 spent ~100us there).
  - Whole pipeline bf16 with fp32 PSUM accumulation; rel err 3.4e-3 vs
    the 2e-2 gate. (fp8+DoubleRow AV was measured ~9us faster but lands at
    rel err 1.95e-2 -- too close to the gate to ship.)
  - Attention runs per head-pair, ACT-bound: per k-chunk the two heads'
    score matmuls (rows 0-63 / 64-127 via auto tile_position) write two
    2-bank psum tiles ping-ponged against [128,1024] exp calls, keeping
    the scalar engine ~88% busy; exp output feeds a 4-slot bf16 ring.
  - AV (with fused ones-column for free rowsums) lags 2 k-chunks and the
    leftover groups are injected between the next pair's first score
    blocks, so neither PE nor ACT idles across pair boundaries (HAM
    re-throttles to 1.2GHz on PE duty dips; v1 ran its whole attention
    phase at half clock because 64-row score matmuls never read as busy).
  - Input DMA is one need-ordered queue (wq,xq | wk,xk | wv,xv | rest):
    splitting across queues just divides bandwidth and stalls the PE.
  - Tail: rowsums packed at 32-aligned partitions, reciprocal as
    exp(-ln(x)) on the idle ACT engine (same table set as the attention
    exps), selector-matmul broadcast, all pipelined by q-half so the out
    projection of the first half overlaps the second half's normalization.
"""
import contextlib

import numpy as np

import bass_rust
import concourse.bass as bass
import concourse.mybir as mybir
import concourse.tile as tile
from concourse.bass_utils import run_bass_kernel_spmd
from concourse.tile import add_dep_helper

F32 = mybir.dt.float32
F32R = mybir.dt.float32r
BF16 = mybir.dt.bfloat16

B, S, D_MODEL = 4, 2048, 512
NUM_HEADS = 8
HEAD_DIM = 64
SQ = S // 2  # queries per core
N_CORES = 8
SCALE = 1.0 / 8.0  # 1/sqrt(HEAD_DIM)
KTILES = S // 128  # 16 k-chunks
H2 = NUM_HEADS // 2  # head pairs
RING = 4  # eh ring depth (k-chunks buffered between exp and AV)

_split_ctr = [0]


def split_waits(nc, max_waits: int = 1):
    """walrus codegen rejects instructions carrying >1 sync wait; move the
    extras onto standalone EventSemaphore instructions on the same engine."""
    for f in nc.m.functions:
        for blk in f.blocks:
            new_insts = []
            changed = False
            for inst in blk.instructions:
                si = inst.sync_info
                if si is not None and si.on_wait and len(si.on_wait) > max_waits:
                    waits = list(si.on_wait)
                    extra, keep = waits[:-max_waits], waits[-max_waits:]
                    for w in extra:
                        _split_ctr[0] += 1
                        ev = mybir.InstEventSemaphore(
                            name=f"I-wsplit-{_split_ctr[0]}", ins=[], outs=[]
                        )
                        ev.engine = inst.engine
                        ev.sync_info = bass_rust.SyncInfo(on_wait=[w], on_update=[])
                        new_insts.append(ev)
                    inst.sync_info = bass_rust.SyncInfo(
                        on_wait=keep, on_update=list(si.on_update)
                    )
                    changed = True
                new_insts.append(inst)
            if changed:
                blk.instructions = new_insts


def build_mha():
    nc = bass.Bass("TRN2", target_bir_lowering=False, debug=False, num_devices=1)

    # host-transposed inputs: xq = q_slice.T [D, SQ], xk = k.T, xv = v.T,
    # weights passed as W.T [d_in, d_out]; all bf16.
    xqd = nc.declare_dram_parameter("xq", [D_MODEL, SQ], BF16, isOutput=False).ap()
    xkd = nc.declare_dram_parameter("xk", [D_MODEL, S], BF16, isOutput=False).ap()
    xvd = nc.declare_dram_parameter("xv", [D_MODEL, S], BF16, isOutput=False).ap()
    wts = {
        n: nc.declare_dram_parameter(n, [D_MODEL, D_MODEL], BF16, isOutput=False).ap()
        for n in ("wq", "wk", "wv", "wo")
    }
    bias = {
        n: nc.declare_dram_parameter(n, [D_MODEL], F32, isOutput=False).ap()
        for n in ("bq", "bk", "bv", "bo")
    }
    outd = nc.declare_dram_parameter("out", [SQ, D_MODEL], F32, isOutput=True).ap()

    # selector rows for the rowsum-reciprocal broadcast: selU row 32t maps
    # head 2t's reciprocal to PE cols 0-63, selD row 32t to cols 64-127
    # (head 2t+1). Rows sit at 32-aligned partitions: DVE/matmul APs must
    # start at a multiple of 32.
    selu_np = np.zeros((128, 128), np.float32)
    seld_np = np.zeros((128, 128), np.float32)
    for t in range(H2):
        selu_np[32 * t, 0:HEAD_DIM] = 1.0
        seld_np[32 * t, HEAD_DIM:128] = 1.0
    selu_dram = nc.inline_tensor(selu_np, name="selu_const")
    seld_dram = nc.inline_tensor(seld_np, name="seld_const")

    with tile.TileContext(nc) as tc, contextlib.ExitStack() as top:
        consts = top.enter_context(tc.tile_pool(name="consts", bufs=1))
        wt_pool = top.enter_context(tc.tile_pool(name="wt", bufs=1))
        proj_out = top.enter_context(tc.tile_pool(name="proj_out", bufs=1))
        epilog = top.enter_context(tc.tile_pool(name="epilog", bufs=1))

        # ---- constants
        bqt = consts.tile([128, 4], F32)
        bkt = consts.tile([128, 4], F32)
        for t_, name in ((bqt, "bq"), (bkt, "bk")):
            nc.gpsimd.dma_start(
                out=t_, in_=bias[name].rearrange("(c p) -> p c", p=128)
            )
        bvb = consts.tile([128, D_MODEL], F32)
        bob = consts.tile([128, D_MODEL], F32)
        ones8 = consts.tile([128, NUM_HEADS], BF16)
        nc.vector.memset(ones8, 1.0)
        selu_f = consts.tile([128, 128], F32)
        seld_f = consts.tile([128, 128], F32)
        selu = consts.tile([128, 128], F32R)
        seld = consts.tile([128, 128], F32R)
        # warm the ACT exp table early (table load ~2.7us)
        dmy = consts.tile([1, 8], F32)
        nc.vector.memset(dmy, 0.0)
        dmy2 = consts.tile([1, 8], F32)
        nc.scalar.activation(dmy2, dmy, mybir.ActivationFunctionType.Exp)

        # ---- persistent weight / activation tiles
        WSB = {
            n: wt_pool.tile([128, 4, D_MODEL], BF16, name=f"w_{n}", tag=f"w_{n}")
            for n in wts
        }
        QT = [proj_out.tile([128, SQ], BF16, name=f"qt_{t}", tag=f"qt_{t}") for t in range(H2)]
        KT = [proj_out.tile([128, S], BF16, name=f"kt_{t}", tag=f"kt_{t}") for t in range(H2)]
        V = [
            proj_out.tile([128, NUM_HEADS, HEAD_DIM + 1], BF16, name=f"v_{sc}", tag=f"v_{sc}")
            for sc in range(KTILES)
        ]
        OU = [epilog.tile([128, SQ], BF16, name=f"ou_{t}", tag=f"ou_{t}") for t in range(H2)]
        # rowsums: head 2t at RSA[32t], head 2t+1 at RSB[32t] (32-aligned)
        RSA = epilog.tile([128, SQ], F32, name="rsa", tag="rsa")
        RSB = epilog.tile([128, SQ], F32, name="rsb", tag="rsb")
        RSAr = epilog.tile([128, SQ], F32R, name="rsar", tag="rsar")
        RSBr = epilog.tile([128, SQ], F32R, name="rsbr", tag="rsbr")
        LNS = epilog.tile([128, SQ], F32, name="lns", tag="lns")
        OMT = [epilog.tile([128, SQ], BF16, name=f"omt_{t}", tag=f"omt_{t}") for t in range(H2)]
        nc.vector.memset(RSA, 1.0)  # unused rows must not hit 1/0
        nc.vector.memset(RSB, 1.0)

        pe_chain = [None]

        def chain(bi):
            if pe_chain[0] is not None:
                add_dep_helper(bi.ins, pe_chain[0].ins, reason="pe-order")
            pe_chain[0] = bi

        # ================= phase 1: projections ===========================
        with (
            tc.tile_pool(name="xin", bufs=1) as xin,
            tc.tile_pool(name="pp", bufs=2, space="PSUM") as pp,
        ):
            xq_sb = xin.tile([128, 4, SQ], BF16, tag="xq")
            xk_sb = xin.tile([128, 4, S], BF16, tag="xk")
            xv_sb = xin.tile([128, 4, S], BF16, tag="xv")
            # one DMA queue, strictly need-ordered: Q-proj inputs first,
            # then K, then V, then wo/consts. Splitting across queues only
            # divides the shared DMA bandwidth and delays the first stream;
            # ordering one queue by first-use keeps the PE fed end to end.
            nc.sync.dma_start(
                out=WSB["wq"], in_=wts["wq"].rearrange("(c p) m -> p c m", p=128)
            )
            for dc in range(4):
                nc.sync.dma_start(
                    out=xq_sb[:, dc, :], in_=xqd[dc * 128 : (dc + 1) * 128, :]
                )
            nc.sync.dma_start(
                out=WSB["wk"], in_=wts["wk"].rearrange("(c p) m -> p c m", p=128)
            )
            for dc in range(4):
                nc.sync.dma_start(
                    out=xk_sb[:, dc, :], in_=xkd[dc * 128 : (dc + 1) * 128, :]
                )
            nc.sync.dma_start(
                out=WSB["wv"], in_=wts["wv"].rearrange("(c p) m -> p c m", p=128)
            )
            for dc in range(4):
                nc.sync.dma_start(
                    out=xv_sb[:, dc, :], in_=xvd[dc * 128 : (dc + 1) * 128, :]
                )
            nc.sync.dma_start(
                out=WSB["wo"], in_=wts["wo"].rearrange("(c p) m -> p c m", p=128)
            )
            # HAM warmer: full-array dummy matmuls bridge the DMA-paced
            # start (first real matmul waits ~16.5us for wq+xq) so the
            # clock monitor sees a busy PE and the projections run at
            # 2.4GHz from their first instruction instead of ramping
            warm_sb = xin.tile([128, 512], BF16, tag="warm")
            nc.vector.memset(warm_sb, 0.5)
            warm_ps = pp.tile([128, 512], F32, tag="pproj")
            for _ in range(20):
                chain(
                    nc.tensor.matmul(
                        warm_ps, warm_sb[:, 0:128], warm_sb, start=True, stop=True
                    )
                )

            def gap_filler():
                # one dummy between DMA-gated projection groups keeps HAM
                # warm while the next group's inputs are still in flight
                gp = pp.tile([128, 512], F32, tag="pproj")
                chain(
                    nc.tensor.matmul(
                        gp, warm_sb[:, 0:128], warm_sb, start=True, stop=True
                    )
                )
            for t_, name in ((bvb, "bv"), (bob, "bo")):
                bsrc = bias[name]
                nc.scalar.dma_start(
                    out=t_,
                    in_=bass.AP(
                        tensor=bsrc.tensor,
                        offset=bsrc.offset,
                        ap=[[0, 128], [1, D_MODEL]],
                    ),
                )
            nc.scalar.dma_start(out=selu_f, in_=selu_dram.ap())
            nc.scalar.dma_start(out=seld_f, in_=seld_dram.ap())
            nc.vector.tensor_copy(selu, selu_f)
            nc.vector.tensor_copy(seld, seld_f)

            # Q^T / K^T projections: [d_out-chunk t partitions, seq free]
            for wname, bt, xsb, dst, slen in (
                ("wq", bqt, xq_sb, QT, SQ),
                ("wk", bkt, xk_sb, KT, S),
            ):
                for t in range(H2):
                    for qc in range(slen // 512):
                        pj = pp.tile([128, 512], F32, tag="pproj")
                        for dc in range(4):
                            chain(
                                nc.tensor.matmul(
                                    pj,
                                    WSB[wname][:, dc, t * 128 : (t + 1) * 128],
                                    xsb[:, dc, qc * 512 : (qc + 1) * 512],
                                    start=(dc == 0),
                                    stop=(dc == 3),
                                )
                            )
                        nc.vector.tensor_scalar_add(
                            dst[t][:, qc * 512 : (qc + 1) * 512],
                            pj,
                            bt[:, t : t + 1],
                        )
                        if wname == "wq" or t < 2:
                            gap_filler()

            # V projection: natural [seq partitions, d_out free] + ones col
            for sc in range(KTILES):
                pj = pp.tile([128, 512], F32, tag="pproj")
                for dc in range(4):
                    chain(
                        nc.tensor.matmul(
                            pj,
                            xv_sb[:, dc, sc * 128 : (sc + 1) * 128],
                            WSB["wv"][:, dc, :],
                            start=(dc == 0),
                            stop=(dc == 3),
                        )
                    )
                pj3 = pj.rearrange("p (h d) -> p h d", h=NUM_HEADS)
                nc.vector.tensor_add(
                    V[sc][:, :, 0:HEAD_DIM],
                    pj3,
                    bvb.rearrange("p (h d) -> p h d", h=NUM_HEADS),
                )
                nc.vector.tensor_copy(
                    V[sc][:, :, HEAD_DIM : HEAD_DIM + 1],
                    ones8.rearrange("p (h o) -> p h o", o=1),
                )

        # ================= phase 2: attention (head pairs) ================
        with (
            tc.tile_pool(name="ehpool", bufs=4) as ehpool,
            tc.tile_pool(name="ps_s", bufs=2, space="PSUM") as ps_s,
            tc.tile_pool(name="ps_o", bufs=2, space="PSUM") as ps_o,
        ):
            # dummy allocation shifts the ps_s rotation so pair 0's first
            # score tile lands on the banks NOT written by the final
            # V-projection group (saves a ~2.4us drain wait at the
            # phase transition; psB runs an exp-slot later, by which time
            # the overlapping banks have drained)
            ps_shift = ps_s.tile([128, SQ], F32, tag="ps", name="ps_shift")
            # pending[0] emits the previous pair's last AV group + finalize;
            # it runs between the next pair's first psA and psB score blocks
            # so the ACT engine never idles across the pair boundary.
            pending = [None]
            for t in range(H2):
                poA = ps_o.tile([HEAD_DIM + 1, SQ], F32, tag="po")
                poB = ps_o.tile([HEAD_DIM + 1, SQ], F32, tag="po")
                ehA = ehpool.tile([128, RING, SQ], BF16, tag="eh")
                ehB = ehpool.tile([128, RING, SQ], BF16, tag="eh")

                def emit_av(kc, t=t, poA=poA, poB=poB, ehA=ehA, ehB=ehB):
                    for ph, po_, eh_ in ((2 * t, poA, ehA), (2 * t + 1, poB, ehB)):
                        for qc in range(2):
                            sl = slice(qc * 512, (qc + 1) * 512)
                            chain(
                                nc.tensor.matmul(
                                    po_[:, sl],
                                    V[kc][:, ph, :],
                                    eh_[:, kc % RING, sl],
                                    start=(kc == 0),
                                    stop=(kc == KTILES - 1),
                                )
                            )

                streams = ((slice(0, HEAD_DIM), ehA), (slice(HEAD_DIM, 128), ehB))
                for kc in range(KTILES):
                    ps_tiles = []
                    for si, (rows, _eh) in enumerate(streams):
                        ps_ = ps_s.tile([128, SQ], F32, tag="ps")
                        ps_tiles.append(ps_)
                        for qc in range(2):
                            sl = slice(qc * 512, (qc + 1) * 512)
                            chain(
                                nc.tensor.matmul(
                                    ps_[:, sl],
                                    KT[t][rows, kc * 128 : (kc + 1) * 128],
                                    QT[t][rows, sl],
                                    start=True,
                                    stop=True,
                                )
                            )
                        # inject the previous pair's tail between the two
                        # score blocks: psB is gated by its exp anyway, so
                        # the AV matmuls fill that PE slot for free
                        if si == 0 and pending[0] is not None:
                            if kc == 0:
                                pending[0][0]()  # prev pair AV(14)
                            elif kc == 1:
                                pending[0][1]()  # prev pair AV(15) + finalize
                    for (rows, eh_), ps_ in zip(streams, ps_tiles):
                        nc.scalar.activation(
                            eh_[:, kc % RING, :],
                            ps_,
                            mybir.ActivationFunctionType.Exp,
                            scale=SCALE,
                        )
                    if kc > 1:
                        emit_av(kc - 2)

                def pair_tail_a(emit_av=emit_av):
                    emit_av(KTILES - 2)

                def pair_tail_b(t=t, poA=poA, poB=poB, emit_av=emit_av):
                    emit_av(KTILES - 1)
                    # rowsum copies first: they gate the tail's Ln/Exp chain
                    nc.vector.tensor_copy(
                        RSA[32 * t : 32 * t + 1, :], poA[HEAD_DIM : HEAD_DIM + 1, :]
                    )
                    nc.vector.tensor_copy(
                        RSB[32 * t : 32 * t + 1, :], poB[HEAD_DIM : HEAD_DIM + 1, :]
                    )
                    nc.vector.tensor_copy(OU[t][0:HEAD_DIM, :], poA[0:HEAD_DIM, :])
                    nc.vector.tensor_copy(OU[t][HEAD_DIM:128, :], poB[0:HEAD_DIM, :])

                pending[0] = (pair_tail_a, pair_tail_b)
            pending[0][0]()
            pending[0][1]()

        # ================= phase 3: normalize + out projection ============
        with (
            tc.tile_pool(name="outsb", bufs=3) as outsb,
            tc.tile_pool(name="ps_n", bufs=3, space="PSUM") as ps_n,
            tc.tile_pool(name="ps_f", bufs=2, space="PSUM") as ps_f,
        ):
            # 1/x as exp(-ln(x)) on the otherwise-idle ACT engine (Ln and Exp
            # share the natural_log_exp table set; DVE reciprocal would cost
            # ~6.5us per tile). The whole tail is pipelined by q-half: the
            # out-projection for q-chunks 0-3 only needs the qc=0 half of
            # the normalization, so it starts while qc=1 is still on ACT.
            Ln, Exp = mybir.ActivationFunctionType.Ln, mybir.ActivationFunctionType.Exp

            for qc in range(2):
                sl = slice(qc * 512, (qc + 1) * 512)
                nc.scalar.activation(LNS[:, sl], RSA[:, sl], Ln)
                nc.scalar.activation(RSAr[:, sl], LNS[:, sl], Exp, scale=-1.0)
                nc.scalar.activation(LNS[:, sl], RSB[:, sl], Ln)
                nc.scalar.activation(RSBr[:, sl], LNS[:, sl], Exp, scale=-1.0)
                for t in range(H2):
                    rsb = ps_n.tile([128, 512], F32, tag="rsb")
                    chain(
                        nc.tensor.matmul(
                            rsb,
                            selu[32 * t : 32 * t + 1, :],
                            RSAr[32 * t : 32 * t + 1, sl],
                            start=True,
                            stop=False,
                            tile_position=(32 * t, 0),
                        )
                    )
                    chain(
                        nc.tensor.matmul(
                            rsb,
                            seld[32 * t : 32 * t + 1, :],
                            RSBr[32 * t : 32 * t + 1, sl],
                            start=False,
                            stop=True,
                            tile_position=(32 * t, 0),
                        )
                    )
                    nc.vector.tensor_mul(OMT[t][:, sl], OU[t][:, sl], rsb)
                for sq in range(qc * 4, qc * 4 + 4):
                    pf = ps_f.tile([128, D_MODEL], F32, tag="pf")
                    for t in range(H2):
                        chain(
                            nc.tensor.matmul(
                                pf,
                                OMT[t][:, sq * 128 : (sq + 1) * 128],
                                WSB["wo"][:, t, :],
                                start=(t == 0),
                                stop=(t == H2 - 1),
                            )
                        )
                    ot = outsb.tile([128, D_MODEL], F32, tag="ot")
                    nc.vector.tensor_add(ot, pf, bob)
                    nc.sync.dma_start(out=outd[sq * 128 : (sq + 1) * 128, :], in_=ot)

    split_waits(nc)
    return nc


_cached_nc = None


def _get_nc():
    global _cached_nc
    if _cached_nc is None:
        _cached_nc = build_mha()
    return _cached_nc


def make_in_maps(q, k, v, Wq, bq, Wk, bk, Wv, bv, Wo, bo):
    import ml_dtypes

    BF = ml_dtypes.bfloat16
    q = np.asarray(q, dtype=np.float32)
    k = np.asarray(k, dtype=np.float32)
    v = np.asarray(v, dtype=np.float32)
    weights = {
        "wq": np.ascontiguousarray(np.asarray(Wq, np.float32).T.astype(BF)),
        "wk": np.ascontiguousarray(np.asarray(Wk, np.float32).T.astype(BF)),
        "wv": np.ascontiguousarray(np.asarray(Wv, np.float32).T.astype(BF)),
        "wo": np.ascontiguousarray(np.asarray(Wo, np.float32).T.astype(BF)),
        "bq": np.ascontiguousarray(np.asarray(bq, np.float32)),
        "bk": np.ascontiguousarray(np.asarray(bk, np.float32)),
        "bv": np.ascontiguousarray(np.asarray(bv, np.float32)),
        "bo": np.ascontiguousarray(np.asarray(bo, np.float32)),
    }
    in_maps = []
    for core in range(N_CORES):
        b, qh = core // 2, core % 2
        in_maps.append(
            {
                "xq": np.ascontiguousarray(
                    q[b, qh * SQ : (qh + 1) * SQ, :].T.astype(BF)
                ),
                "xk": np.ascontiguousarray(k[b].T.astype(BF)),
                "xv": np.ascontiguousarray(v[b].T.astype(BF)),
                **weights,
            }
        )
    return in_maps


def kernel(q, k, v, mask, Wq, bq, Wk, bk, Wv, bv, Wo, bo, **_unused):
    in_maps = make_in_maps(q, k, v, Wq, bq, Wk, bk, Wv, bv, Wo, bo)
    nc = _get_nc()
    res = run_bass_kernel_spmd(nc, in_maps, list(range(N_CORES)))
    out = np.empty((B, S, D_MODEL), dtype=np.float32)
    for core in range(N_CORES):
        b, qh = core // 2, core % 2
        out[b, qh * SQ : (qh + 1) * SQ, :] = res.results[core]["out"]
    return out
